# revision 18
# baseline (speedup 1.0000x reference)
"""GPR surrogate prediction kernel for Trainium2 (8 NeuronCores, Bass/Tile).

Computes pred = K_star @ alpha where K_star = exp(-||x_m - xtrain_n||^2 / 2).

Factored form (exact in real arithmetic):
    pred[m] = exp(-sq1[m]/2) * sum_n exp(x_m . xt_n + B) * arB[n],
    arB[n] = alpha[n] * exp(-sq2[n]/2),  B a constant rebias so the
    per-element exp stays inside bf16 range; the exp(-B) factor is folded
    into the final per-row scale  sc[m] = exp(B - sq1[m]/2).

With randn inputs at D=256, sq2[n] ~ 256 +- 21, so ar[n] underflows fp32 for
all but a handful of columns (58 of 8192 for the reference data).  Columns
with ar[n] == 0.0f contribute *exactly* zero to the fp32 accumulation, so the
host prunes them and the device computes only the surviving columns (padded
to NZCAP).  If more than NZCAP columns survive (never for randn-scale data),
the kernel runs multiple passes and sums the partial results on host, so the
algorithm stays correct for arbitrary inputs.

Per-core device program, transposed layout [nz=128 partitions, m=512 free]:
  - TensorE: dot[n, m] = Xt_nz @ X_c^T                  (bf16, fp32 PSUM)
  - ScalarE: kw[n, m] = exp(dot + B)                    (one ACT, [128, 512])
  - TensorE: s[i, mt] = sum_n ar[n] * kw[n, mt*128+i]   (four F=1 matmuls,
             fp32 PSUM accumulation over partitions, one shared LDWEIGHTS)
  - VectorE: y = s * sc  (per-row scale, PSUM -> SBUF); one 2KB DMA out.

Fixed costs dominate at this size: the NEFF/bass preamble+epilogue is ~11us,
the exp table load (~1.3us) runs on the ACT queue concurrently with the
input DMAs, and each dma_start costs ~0.65us of descriptor generation on its
issuing queue (xt goes on the Scalar HWDGE queue, wt + y on Sync).
"""

import functools

import ml_dtypes
import numpy as np

M, N, D = 4096, 8192, 256
NCORES = 8
P = 128
MC = M // NCORES          # 512 query rows per core
MT = MC // P              # 4 m-tiles per core
DCH = D // P              # 2 contraction chunks
NH = 2                    # m-halves (pipelined DMA -> matmul -> exp)
MCH = MC // NH            # 256 query rows per half
NZCAP = P                 # pruned columns per pass
BIAS = -46.0              # exp rebias: keeps exp(dot+B) in bf16 range

BF16 = ml_dtypes.bfloat16
WCOLS = 2 * P + 1 + MT    # w d0 | w d1 | ar | sc


@functools.lru_cache(maxsize=1)
def _build():
    import concourse.bacc as bacc
    import concourse.mybir as mybir
    import concourse.tile as tile

    fp32 = mybir.dt.float32
    bf16 = mybir.dt.bfloat16

    nc = bacc.Bacc(
        "TRN2",
        target_bir_lowering=False,
        debug=False,
        enable_asserts=False,
        num_devices=NCORES,
        enable_partition_id=False,
        monotonic_sem_count=0,
        detect_race_conditions=False,
    )

    # wt cols 0:128 / 128:256 = Xt_nz contraction chunks (partition = feature
    # within chunk, col = nz index); col 256 = arB[nz] (partition = nz index);
    # cols 257:261 = sc[m] per m-tile (partition = m within tile).
    wt = nc.dram_tensor("wt", [P, WCOLS], bf16, kind="ExternalInput").ap()
    xt = nc.dram_tensor("xt", [P, NH, DCH, MCH], bf16, kind="ExternalInput").ap()
    y = nc.dram_tensor("y", [P, MT], fp32, kind="ExternalOutput").ap()

    with tile.TileContext(nc) as tc:
        with (
            tc.tile_pool(name="const", bufs=1) as cpool,
            tc.tile_pool(name="psum", bufs=2, space="PSUM") as ppool,
        ):
            wt_sb = cpool.tile([P, WCOLS], bf16, name="wt_sb")
            xt_sb = cpool.tile([P, NH, DCH, MCH], bf16, name="xt_sb")
            kw = cpool.tile([P, MC], bf16, name="kw")
            y_sb = cpool.tile([P, MT], fp32, name="y_sb")
            bias_sb = cpool.tile([P, 1], fp32, name="bias_sb")
            nc.vector.memset(bias_sb[:], BIAS)

            # xt is split into two m-halves, one per HWDGE queue, so the
            # first half's matmul+exp overlap the second half's transfer.
            # The exp table load (~1.3us) follows xt-h0 on the Scalar queue.
            nc.scalar.dma_start(xt_sb[:, 0], xt[:, 0])
            nc.sync.dma_start(wt_sb[:], wt[:])
            nc.sync.dma_start(xt_sb[:, 1], xt[:, 1])

            ps1 = ppool.tile([P, NH, MCH], fp32, name="ps1")
            for h in range(NH):
                for d in range(DCH):
                    nc.tensor.matmul(
                        ps1[:, h],
                        lhsT=wt_sb[:, d * P : (d + 1) * P],
                        rhs=xt_sb[:, h, d],
                        start=(d == 0),
                        stop=(d == DCH - 1),
                    )
                nc.scalar.activation(
                    kw[:, h * MCH : (h + 1) * MCH], ps1[:, h],
                    mybir.ActivationFunctionType.Exp,
                    bias=bias_sb[:], scale=1.0,
                )

            # Stage 2: s[i, mt] = sum_n arB[n] * kw[n, mt*P + i].
            ps2 = ppool.tile([P, MT], fp32, name="ps2")
            for mt in range(MT):
                nc.tensor.matmul(
                    ps2[:, mt : mt + 1],
                    lhsT=kw[:, mt * P : (mt + 1) * P],
                    rhs=wt_sb[:, 2 * P : 2 * P + 1],
                    start=True,
                    stop=True,
                )
            # y[i, mt] = s[i, mt] * sc[mt*P + i]
            nc.vector.tensor_mul(y_sb[:], ps2[:], wt_sb[:, 2 * P + 1 :])
            nc.sync.dma_start(y[:], y_sb[:])

    nc.compile()
    return nc


def _prep_inputs(X, X_train, alpha):
    """Host-side prep. Returns a list of per-pass in_maps (usually one pass).

    O((M+N)*D) work: casts, transposes, norms, and the nonzero-column scan.
    """
    X = np.asarray(X, dtype=np.float32)
    X_train = np.asarray(X_train, dtype=np.float32)
    alpha = np.asarray(alpha, dtype=np.float32).reshape(-1)

    sq1 = np.sum(X.astype(np.float64) ** 2, axis=1)        # [M]
    sq2 = np.sum(X_train.astype(np.float64) ** 2, axis=1)  # [N]

    # ar = alpha * exp(-||xtrain||^2/2); fp64 -> fp32 cast underflows to 0
    # exactly where the reference's fp32 exp does.  Zero columns contribute
    # exactly 0 to the fp32 accumulation, so only nonzero ones are computed.
    ar = (alpha.astype(np.float64) * np.exp(-sq2 / 2.0)).astype(np.float32)
    nz = np.flatnonzero(ar)

    # Per-row scale exp(B - sq1/2), [M] fp32 (cast to bf16 in the pack).
    sc = np.exp(BIAS - sq1 / 2.0).astype(np.float32)

    # xt[p, h, d, i] = X[c*MC + h*MCH + i, d*P + p]: per-core [P, NH, DCH, MCH]
    xt_full = np.ascontiguousarray(
        X.T.astype(BF16).reshape(DCH, P, M).transpose(1, 0, 2)
    )  # [P, DCH, M]

    npass = max(1, -(-len(nz) // NZCAP))
    passes = []
    for k in range(npass):
        sel = nz[k * NZCAP : (k + 1) * NZCAP]
        wt_np = np.zeros((P, WCOLS), dtype=BF16)
        # wt[p, d*P + j] = Xt[sel_j, d*P + p]
        wtT = X_train[sel].astype(BF16).T          # [D, |sel|]
        wt_np[:, 0 : len(sel)] = wtT[0:P]
        wt_np[:, P : P + len(sel)] = wtT[P : 2 * P]
        wt_np[: len(sel), 2 * P] = ar[sel].astype(BF16)

        in_maps = []
        for c in range(NCORES):
            ms = slice(c * MC, (c + 1) * MC)
            wt_c = wt_np.copy()
            # sc[p, mt] = exp(B - sq1[c*MC + mt*P + p]/2)
            wt_c[:, 2 * P + 1 :] = sc[ms].reshape(MT, P).T.astype(BF16)
            # [P, DCH, MC] -> [P, NH, DCH, MCH]
            xt_c = np.ascontiguousarray(
                xt_full[:, :, ms].reshape(P, DCH, NH, MCH).transpose(0, 2, 1, 3)
            )
            in_maps.append({"wt": wt_c, "xt": xt_c})
        passes.append(in_maps)
    return passes


LAST_RES = None


def kernel(X, X_train, alpha):
    from concourse import bass_utils

    nc = _build()
    passes = _prep_inputs(X, X_train, alpha)

    out = np.zeros((M, 1), dtype=np.float64)
    global LAST_RES
    for in_maps in passes:
        res = bass_utils.run_bass_kernel_spmd(
            nc, in_maps, core_ids=list(range(NCORES))
        ).results
        LAST_RES = res
        for c in range(NCORES):
            yc = res[c]["y"]  # [P, MT]; col mt holds rows c*MC + mt*P .. +P
            out[c * MC : (c + 1) * MC, 0] += yc.T.reshape(MC).astype(np.float64)
    return out.astype(np.float32)


if __name__ == "__main__":
    rng = np.random.default_rng(0)
    X = rng.standard_normal((M, D), dtype=np.float32)
    Xt = rng.standard_normal((N, D), dtype=np.float32)
    a = rng.standard_normal((N, 1), dtype=np.float32)
    out = kernel(X=X, X_train=Xt, alpha=a)
    print("out", out.shape, out.dtype, "nonzero:", np.count_nonzero(out))


# revision 22
# speedup vs baseline: 1.0404x; 1.0404x over previous
"""GPR surrogate prediction kernel for Trainium2 (8 NeuronCores, Bass/Tile).

Computes pred = K_star @ alpha where K_star = exp(-||x_m - xtrain_n||^2 / 2).

Factored form (exact in real arithmetic):
    pred[m] = exp(-sq1[m]/2) * sum_n exp(x_m . xt_n + B) * arB[n],
    arB[n] = alpha[n] * exp(-sq2[n]/2),  B a constant rebias so the
    per-element exp stays inside bf16 range; the exp(-B) factor is folded
    into the final per-row scale  sc[m] = exp(B - sq1[m]/2).

With randn inputs at D=256, sq2[n] ~ 256 +- 21, so ar[n] underflows fp32 for
all but a handful of columns (58 of 8192 for the reference data).  Columns
with ar[n] == 0.0f contribute *exactly* zero to the fp32 accumulation, so the
host prunes them and the device computes only the surviving columns (padded
to NZCAP).  If more than NZCAP columns survive (never for randn-scale data),
the kernel runs multiple passes and sums the partial results on host, so the
algorithm stays correct for arbitrary inputs.

Per-core device program, transposed layout [nz=128 partitions, m=512 free]:
  - TensorE: dot[n, m] = Xt_nz @ X_c^T                  (bf16, fp32 PSUM)
  - ScalarE: kw[n, m] = exp(dot + B)                    (one ACT, [128, 512])
  - TensorE: s[i, mt] = sum_n ar[n] * kw[n, mt*128+i]   (four F=1 matmuls,
             fp32 PSUM accumulation over partitions, one shared LDWEIGHTS)
  - VectorE: y = s * sc  (per-row scale, PSUM -> SBUF); one 2KB DMA out.

Fixed costs dominate at this size: the NEFF/bass preamble+epilogue is ~11us,
the exp table load (~1.3us) runs on the ACT queue concurrently with the
input DMAs, and each dma_start costs ~0.65us of descriptor generation on its
issuing queue (xt goes on the Scalar HWDGE queue, wt + y on Sync).
"""

import functools

import ml_dtypes
import numpy as np

M, N, D = 4096, 8192, 256
NCORES = 8
P = 128
MC = M // NCORES          # 512 query rows per core
MT = MC // P              # 4 m-tiles per core
DCH = D // P              # 2 contraction chunks
NH = 2                    # m-halves (pipelined DMA -> matmul -> exp)
MCH = MC // NH            # 256 query rows per half
NZCAP = P                 # pruned columns per pass
BIAS = -46.0              # exp rebias: keeps exp(dot+B) in bf16 range

BF16 = ml_dtypes.bfloat16
WCOLS = 2 * P + 1 + MT    # w d0 | w d1 | ar | sc


@functools.lru_cache(maxsize=1)
def _build():
    import concourse.bacc as bacc
    import concourse.mybir as mybir
    import concourse.tile as tile

    fp32 = mybir.dt.float32
    bf16 = mybir.dt.bfloat16

    nc = bacc.Bacc(
        "TRN2",
        target_bir_lowering=False,
        debug=False,
        enable_asserts=False,
        num_devices=NCORES,
        enable_partition_id=False,
        monotonic_sem_count=0,
        detect_race_conditions=False,
    )

    # wt cols 0:128 / 128:256 = Xt_nz contraction chunks (partition = feature
    # within chunk, col = nz index); col 256 = arB[nz] (partition = nz index);
    # cols 257:261 = sc[m] per m-tile (partition = m within tile).
    wt = nc.dram_tensor("wt", [P, WCOLS], bf16, kind="ExternalInput").ap()
    xt = nc.dram_tensor("xt", [P, NH, DCH, MCH], bf16, kind="ExternalInput").ap()
    y = nc.dram_tensor("y", [P, MT], fp32, kind="ExternalOutput").ap()

    with tile.TileContext(nc) as tc:
        with (
            tc.tile_pool(name="const", bufs=1) as cpool,
            tc.tile_pool(name="psum", bufs=2, space="PSUM") as ppool,
            tc.tile_pool(name="psum2", bufs=1, space="PSUM") as ppool2,
        ):
            wt_sb = cpool.tile([P, WCOLS], bf16, name="wt_sb")
            xt_sb = cpool.tile([P, NH, DCH, MCH], bf16, name="xt_sb")
            kw = cpool.tile([P, MC], bf16, name="kw")
            y_sb = cpool.tile([P, MT], fp32, name="y_sb")
            bias_sb = cpool.tile([P, 1], fp32, name="bias_sb")
            nc.vector.memset(bias_sb[:], BIAS)

            # xt is split into two m-halves, one per HWDGE queue, so the
            # first half's matmul+exp overlap the second half's transfer.
            # The exp table load (~1.3us) follows xt-h0 on the Scalar queue.
            nc.scalar.dma_start(xt_sb[:, 0], xt[:, 0])
            nc.sync.dma_start(wt_sb[:], wt[:])
            nc.sync.dma_start(xt_sb[:, 1], xt[:, 1])

            for h in range(NH):
                # One PSUM tile per half: sharing a bank would serialize
                # half 1's matmuls behind half 0's exp (acc-group reset).
                ps1 = ppool.tile([P, MCH], fp32, name=f"ps1_{h}")
                for d in range(DCH):
                    nc.tensor.matmul(
                        ps1[:],
                        lhsT=wt_sb[:, d * P : (d + 1) * P],
                        rhs=xt_sb[:, h, d],
                        start=(d == 0),
                        stop=(d == DCH - 1),
                    )
                nc.scalar.activation(
                    kw[:, h * MCH : (h + 1) * MCH], ps1[:],
                    mybir.ActivationFunctionType.Exp,
                    bias=bias_sb[:], scale=1.0,
                )

            # Stage 2: s[i, mt] = sum_n arB[n] * kw[n, mt*P + i].
            ps2 = ppool2.tile([P, MT], fp32, name="ps2")
            for mt in range(MT):
                nc.tensor.matmul(
                    ps2[:, mt : mt + 1],
                    lhsT=kw[:, mt * P : (mt + 1) * P],
                    rhs=wt_sb[:, 2 * P : 2 * P + 1],
                    start=True,
                    stop=True,
                )
            # y[i, mt] = s[i, mt] * sc[mt*P + i]
            nc.vector.tensor_mul(y_sb[:], ps2[:], wt_sb[:, 2 * P + 1 :])
            nc.sync.dma_start(y[:], y_sb[:])

    nc.compile()
    return nc


def _prep_inputs(X, X_train, alpha):
    """Host-side prep. Returns a list of per-pass in_maps (usually one pass).

    O((M+N)*D) work: casts, transposes, norms, and the nonzero-column scan.
    """
    X = np.asarray(X, dtype=np.float32)
    X_train = np.asarray(X_train, dtype=np.float32)
    alpha = np.asarray(alpha, dtype=np.float32).reshape(-1)

    sq1 = np.sum(X.astype(np.float64) ** 2, axis=1)        # [M]
    sq2 = np.sum(X_train.astype(np.float64) ** 2, axis=1)  # [N]

    # ar = alpha * exp(-||xtrain||^2/2); fp64 -> fp32 cast underflows to 0
    # exactly where the reference's fp32 exp does.  Zero columns contribute
    # exactly 0 to the fp32 accumulation, so only nonzero ones are computed.
    ar = (alpha.astype(np.float64) * np.exp(-sq2 / 2.0)).astype(np.float32)
    nz = np.flatnonzero(ar)

    # Per-row scale exp(B - sq1/2), [M] fp32 (cast to bf16 in the pack).
    sc = np.exp(BIAS - sq1 / 2.0).astype(np.float32)

    # xt[p, h, d, i] = X[c*MC + h*MCH + i, d*P + p]: per-core [P, NH, DCH, MCH]
    xt_full = np.ascontiguousarray(
        X.T.astype(BF16).reshape(DCH, P, M).transpose(1, 0, 2)
    )  # [P, DCH, M]

    npass = max(1, -(-len(nz) // NZCAP))
    passes = []
    for k in range(npass):
        sel = nz[k * NZCAP : (k + 1) * NZCAP]
        wt_np = np.zeros((P, WCOLS), dtype=BF16)
        # wt[p, d*P + j] = Xt[sel_j, d*P + p]
        wtT = X_train[sel].astype(BF16).T          # [D, |sel|]
        wt_np[:, 0 : len(sel)] = wtT[0:P]
        wt_np[:, P : P + len(sel)] = wtT[P : 2 * P]
        wt_np[: len(sel), 2 * P] = ar[sel].astype(BF16)

        in_maps = []
        for c in range(NCORES):
            ms = slice(c * MC, (c + 1) * MC)
            wt_c = wt_np.copy()
            # sc[p, mt] = exp(B - sq1[c*MC + mt*P + p]/2)
            wt_c[:, 2 * P + 1 :] = sc[ms].reshape(MT, P).T.astype(BF16)
            # [P, DCH, MC] -> [P, NH, DCH, MCH]
            xt_c = np.ascontiguousarray(
                xt_full[:, :, ms].reshape(P, DCH, NH, MCH).transpose(0, 2, 1, 3)
            )
            in_maps.append({"wt": wt_c, "xt": xt_c})
        passes.append(in_maps)
    return passes


LAST_RES = None


def kernel(X, X_train, alpha):
    from concourse import bass_utils

    nc = _build()
    passes = _prep_inputs(X, X_train, alpha)

    out = np.zeros((M, 1), dtype=np.float64)
    global LAST_RES
    for in_maps in passes:
        res = bass_utils.run_bass_kernel_spmd(
            nc, in_maps, core_ids=list(range(NCORES))
        ).results
        LAST_RES = res
        for c in range(NCORES):
            yc = res[c]["y"]  # [P, MT]; col mt holds rows c*MC + mt*P .. +P
            out[c * MC : (c + 1) * MC, 0] += yc.T.reshape(MC).astype(np.float64)
    return out.astype(np.float32)


if __name__ == "__main__":
    rng = np.random.default_rng(0)
    X = rng.standard_normal((M, D), dtype=np.float32)
    Xt = rng.standard_normal((N, D), dtype=np.float32)
    a = rng.standard_normal((N, 1), dtype=np.float32)
    out = kernel(X=X, X_train=Xt, alpha=a)
    print("out", out.shape, out.dtype, "nonzero:", np.count_nonzero(out))


# revision 23
# speedup vs baseline: 1.0504x; 1.0096x over previous
"""GPR surrogate prediction kernel for Trainium2 (8 NeuronCores, Bass/Tile).

Computes pred = K_star @ alpha where K_star = exp(-||x_m - xtrain_n||^2 / 2).

Factored form (exact in real arithmetic):
    pred[m] = exp(-sq1[m]/2) * sum_n exp(x_m . xt_n + B) * arB[n],
    arB[n] = alpha[n] * exp(-sq2[n]/2),  B a constant rebias so the
    per-element exp stays inside bf16 range; the exp(-B) factor is folded
    into the final per-row scale  sc[m] = exp(B - sq1[m]/2).

With randn inputs at D=256, sq2[n] ~ 256 +- 21, so ar[n] underflows fp32 for
all but a handful of columns (58 of 8192 for the reference data).  Columns
with ar[n] == 0.0f contribute *exactly* zero to the fp32 accumulation, so the
host prunes them and the device computes only the surviving columns (padded
to NZCAP).  If more than NZCAP columns survive (never for randn-scale data),
the kernel runs multiple passes and sums the partial results on host, so the
algorithm stays correct for arbitrary inputs.

Per-core device program, transposed layout [nz=128 partitions, m=512 free]:
  - TensorE: dot[n, m] = Xt_nz @ X_c^T                  (bf16, fp32 PSUM)
  - ScalarE: kw[n, m] = exp(dot + B)                    (one ACT, [128, 512])
  - TensorE: s[i, mt] = sum_n ar[n] * kw[n, mt*128+i]   (four F=1 matmuls,
             fp32 PSUM accumulation over partitions, one shared LDWEIGHTS)
  - VectorE: y = s * sc  (per-row scale, PSUM -> SBUF); one 2KB DMA out.

Fixed costs dominate at this size: the NEFF/bass preamble+epilogue is ~11us,
the exp table load (~1.3us) runs on the ACT queue concurrently with the
input DMAs, and each dma_start costs ~0.65us of descriptor generation on its
issuing queue (xt goes on the Scalar HWDGE queue, wt + y on Sync).
"""

import functools

import ml_dtypes
import numpy as np

M, N, D = 4096, 8192, 256
NCORES = 8
P = 128
MC = M // NCORES          # 512 query rows per core
MT = MC // P              # 4 m-tiles per core
DCH = D // P              # 2 contraction chunks
NH = 2                    # m-halves (pipelined DMA -> matmul -> exp)
MCH = MC // NH            # 256 query rows per half
NZCAP = P                 # pruned columns per pass
BIAS = -46.0              # exp rebias: keeps exp(dot+B) in bf16 range

BF16 = ml_dtypes.bfloat16
WCOLS = 2 * P + 1 + MT    # w d0 | w d1 | ar | sc


@functools.lru_cache(maxsize=1)
def _build():
    import concourse.bacc as bacc
    import concourse.mybir as mybir
    import concourse.tile as tile

    fp32 = mybir.dt.float32
    bf16 = mybir.dt.bfloat16

    nc = bacc.Bacc(
        "TRN2",
        target_bir_lowering=False,
        debug=False,
        enable_asserts=False,
        num_devices=NCORES,
        enable_partition_id=False,
        monotonic_sem_count=0,
        detect_race_conditions=False,
    )

    # wt cols 0:128 / 128:256 = Xt_nz contraction chunks (partition = feature
    # within chunk, col = nz index); col 256 = arB[nz] (partition = nz index);
    # cols 257:261 = sc[m] per m-tile (partition = m within tile).
    wt = nc.dram_tensor("wt", [P, WCOLS], bf16, kind="ExternalInput").ap()
    xt = nc.dram_tensor("xt", [P, NH, DCH, MCH], bf16, kind="ExternalInput").ap()
    y = nc.dram_tensor("y", [P, MT], fp32, kind="ExternalOutput").ap()

    with tile.TileContext(nc) as tc:
        with (
            tc.tile_pool(name="const", bufs=1) as cpool,
            tc.tile_pool(name="psum", bufs=2, space="PSUM") as ppool,
            tc.tile_pool(name="psum2", bufs=1, space="PSUM") as ppool2,
        ):
            wt_sb = cpool.tile([P, WCOLS], bf16, name="wt_sb")
            xt_sb = cpool.tile([P, NH, DCH, MCH], bf16, name="xt_sb")
            kw = cpool.tile([P, MC], bf16, name="kw")
            y_sb = cpool.tile([P, MT], fp32, name="y_sb")
            bias_sb = cpool.tile([P, 1], fp32, name="bias_sb")
            nc.vector.memset(bias_sb[:], BIAS)

            # Scalar issues wt (needed first, for LDWEIGHTS) and then its
            # exp table load (~1.3us); Sync issues xt as two m-halves so the
            # first half's matmuls+exp overlap the second half's transfer.
            nc.scalar.dma_start(wt_sb[:], wt[:])
            nc.sync.dma_start(xt_sb[:, 0], xt[:, 0])
            nc.sync.dma_start(xt_sb[:, 1], xt[:, 1])

            for h in range(NH):
                # One PSUM tile per half: sharing a bank would serialize
                # half 1's matmuls behind half 0's exp (acc-group reset).
                ps1 = ppool.tile([P, MCH], fp32, name=f"ps1_{h}")
                for d in range(DCH):
                    nc.tensor.matmul(
                        ps1[:],
                        lhsT=wt_sb[:, d * P : (d + 1) * P],
                        rhs=xt_sb[:, h, d],
                        start=(d == 0),
                        stop=(d == DCH - 1),
                    )
                nc.scalar.activation(
                    kw[:, h * MCH : (h + 1) * MCH], ps1[:],
                    mybir.ActivationFunctionType.Exp,
                    bias=bias_sb[:], scale=1.0,
                )

            # Stage 2: s[i, mt] = sum_n arB[n] * kw[n, mt*P + i].
            ps2 = ppool2.tile([P, MT], fp32, name="ps2")
            for mt in range(MT):
                nc.tensor.matmul(
                    ps2[:, mt : mt + 1],
                    lhsT=kw[:, mt * P : (mt + 1) * P],
                    rhs=wt_sb[:, 2 * P : 2 * P + 1],
                    start=True,
                    stop=True,
                )
            # y[i, mt] = s[i, mt] * sc[mt*P + i]
            nc.vector.tensor_mul(y_sb[:], ps2[:], wt_sb[:, 2 * P + 1 :])
            nc.sync.dma_start(y[:], y_sb[:])

    nc.compile()
    return nc


def _prep_inputs(X, X_train, alpha):
    """Host-side prep. Returns a list of per-pass in_maps (usually one pass).

    O((M+N)*D) work: casts, transposes, norms, and the nonzero-column scan.
    """
    X = np.asarray(X, dtype=np.float32)
    X_train = np.asarray(X_train, dtype=np.float32)
    alpha = np.asarray(alpha, dtype=np.float32).reshape(-1)

    sq1 = np.sum(X.astype(np.float64) ** 2, axis=1)        # [M]
    sq2 = np.sum(X_train.astype(np.float64) ** 2, axis=1)  # [N]

    # ar = alpha * exp(-||xtrain||^2/2); fp64 -> fp32 cast underflows to 0
    # exactly where the reference's fp32 exp does.  Zero columns contribute
    # exactly 0 to the fp32 accumulation, so only nonzero ones are computed.
    ar = (alpha.astype(np.float64) * np.exp(-sq2 / 2.0)).astype(np.float32)
    nz = np.flatnonzero(ar)

    # Per-row scale exp(B - sq1/2), [M] fp32 (cast to bf16 in the pack).
    sc = np.exp(BIAS - sq1 / 2.0).astype(np.float32)

    # xt[p, h, d, i] = X[c*MC + h*MCH + i, d*P + p]: per-core [P, NH, DCH, MCH]
    xt_full = np.ascontiguousarray(
        X.T.astype(BF16).reshape(DCH, P, M).transpose(1, 0, 2)
    )  # [P, DCH, M]

    npass = max(1, -(-len(nz) // NZCAP))
    passes = []
    for k in range(npass):
        sel = nz[k * NZCAP : (k + 1) * NZCAP]
        wt_np = np.zeros((P, WCOLS), dtype=BF16)
        # wt[p, d*P + j] = Xt[sel_j, d*P + p]
        wtT = X_train[sel].astype(BF16).T          # [D, |sel|]
        wt_np[:, 0 : len(sel)] = wtT[0:P]
        wt_np[:, P : P + len(sel)] = wtT[P : 2 * P]
        wt_np[: len(sel), 2 * P] = ar[sel].astype(BF16)

        in_maps = []
        for c in range(NCORES):
            ms = slice(c * MC, (c + 1) * MC)
            wt_c = wt_np.copy()
            # sc[p, mt] = exp(B - sq1[c*MC + mt*P + p]/2)
            wt_c[:, 2 * P + 1 :] = sc[ms].reshape(MT, P).T.astype(BF16)
            # [P, DCH, MC] -> [P, NH, DCH, MCH]
            xt_c = np.ascontiguousarray(
                xt_full[:, :, ms].reshape(P, DCH, NH, MCH).transpose(0, 2, 1, 3)
            )
            in_maps.append({"wt": wt_c, "xt": xt_c})
        passes.append(in_maps)
    return passes


LAST_RES = None


def kernel(X, X_train, alpha):
    from concourse import bass_utils

    nc = _build()
    passes = _prep_inputs(X, X_train, alpha)

    out = np.zeros((M, 1), dtype=np.float64)
    global LAST_RES
    for in_maps in passes:
        res = bass_utils.run_bass_kernel_spmd(
            nc, in_maps, core_ids=list(range(NCORES))
        ).results
        LAST_RES = res
        for c in range(NCORES):
            yc = res[c]["y"]  # [P, MT]; col mt holds rows c*MC + mt*P .. +P
            out[c * MC : (c + 1) * MC, 0] += yc.T.reshape(MC).astype(np.float64)
    return out.astype(np.float32)


if __name__ == "__main__":
    rng = np.random.default_rng(0)
    X = rng.standard_normal((M, D), dtype=np.float32)
    Xt = rng.standard_normal((N, D), dtype=np.float32)
    a = rng.standard_normal((N, 1), dtype=np.float32)
    out = kernel(X=X, X_train=Xt, alpha=a)
    print("out", out.shape, out.dtype, "nonzero:", np.count_nonzero(out))
